# revision 17
# baseline (speedup 1.0000x reference)
"""Trainium2 Bass kernel for a 2-layer LSTM decoder (B=128, T=32, F=2048,
E=512, H=1024, V=10000), gate-TENSOR-parallel across 8 NeuronCores.

Sharding (vs. the data-parallel hint): each core owns a 1/8 slice of the
hidden dim (HSL=128) => a 512-col gate slice [i|f|o|g] of every LSTM
weight, the matching slice of the cell state, and a 1250-col vocab slice
of the FC layer. The full batch B=128 is the matmul stationary operand on
every core, so the PE array runs at full width (vs 16/128 for DP) and the
serial weight-stream cost of the recurrence is sharded 8x.

Per step the cores exchange hidden-state slices with ONE 8-way AllGather
of [h0T(t) ; h1T(t-1)] (layer-1 runs one step skewed so both slices ride
the same collective; flat 1-D collective APs give rank-concat order).
The ~13us collective latency per step is hidden behind FC(t-2) matmuls,
the embedding gather/transpose for step t+2, and X0(t+2) = emb @ wih0
precompute, all of which are emitted after the collective doorbell so
the PE never idles (idle PE also downclocks, making restarts slower).

All matmul operands fp16 (1 PE row/cycle), fp32 PSUM + cell state.
Biases fold in via K=1 ones-row matmuls.
"""

import numpy as np

import concourse.bass as bass
import concourse.mybir as mybir
from concourse import bacc
from concourse.bass_utils import run_bass_kernel_spmd
from concourse.masks import make_identity
from concourse.tile import TileContext

P = 128
NCORES = 8
B, T, F, E, H, L, V = 128, 32, 2048, 512, 1024, 2, 10000
G = 4 * H
TB = T * B                 # 4096 output rows per core, t-major
HSL = H // NCORES          # 128 hidden cols per core
GSL = 4 * HSL              # 512 gate cols per core (i,f,o,g x 128)
VSL = V // NCORES          # 1250 vocab cols per core
KF, KE, KH = F // P, E // P, H // P      # 16, 4, 8
F16 = mybir.dt.float16
F32 = mybir.dt.float32
RG = [list(range(NCORES))]

_cache = {}


def _build_nc():
    nc = bacc.Bacc("TRN2", target_bir_lowering=False, debug=False,
                   enable_asserts=False, num_devices=NCORES)

    def din(name, shape, dt=F16):
        return nc.dram_tensor(name, shape, dt, kind="ExternalInput").ap()

    table = din("table", [V, E])
    emb_idx = din("emb_idx", [TB, 1], mybir.dt.int32)
    featT = din("featT", [F, B])
    initw_h = din("initw_h", [F, 2 * HSL])
    initw_c = din("initw_c", [F, 2 * HSL])
    initb_h = din("initb_h", [1, 2 * HSL])
    initb_c = din("initb_c", [1, 2 * HSL])
    wih0T = din("wih0T", [E, GSL])
    whh0T = din("whh0T", [H, GSL])
    wih1T = din("wih1T", [H, GSL])
    whh1T = din("whh1T", [H, GSL])
    bsum0 = din("bsum0", [1, GSL])
    bsum1 = din("bsum1", [1, GSL])
    fcwT = din("fcwT", [H, VSL])
    fcb_rep = din("fcb_rep", [P, VSL], F32)
    out = nc.dram_tensor("out", [TB, VSL], F32, kind="ExternalOutput").ap()

    featT_v = featT.rearrange("(k p) b -> p k b", p=P)
    initw_h_v = initw_h.rearrange("(k p) n -> p k n", p=P)
    initw_c_v = initw_c.rearrange("(k p) n -> p k n", p=P)
    wih0T_v = wih0T.rearrange("(k p) g -> p k g", p=P)
    whh0T_v = whh0T.rearrange("(k p) g -> p k g", p=P)
    wih1T_v = wih1T.rearrange("(k p) g -> p k g", p=P)
    whh1T_v = whh1T.rearrange("(k p) g -> p k g", p=P)
    fcwT_v = fcwT.rearrange("(k p) v -> p k v", p=P)
    idx_v = emb_idx.rearrange("(g p) one -> p g one", p=P)

    SIG = mybir.ActivationFunctionType.Sigmoid
    TANH = mybir.ActivationFunctionType.Tanh

    with TileContext(nc) as tc, \
         tc.tile_pool(name="const", bufs=1) as constp, \
         tc.tile_pool(name="resident", bufs=1) as resp, \
         tc.tile_pool(name="state", bufs=1) as statep, \
         tc.tile_pool(name="ccd", bufs=3, space="DRAM") as ccp, \
         tc.tile_pool(name="stg", bufs=3) as stgp, \
         tc.tile_pool(name="hg", bufs=3) as hgp, \
         tc.tile_pool(name="embg", bufs=4) as embg, \
         tc.tile_pool(name="embt", bufs=3) as embtp, \
         tc.tile_pool(name="act", bufs=2) as actp, \
         tc.tile_pool(name="hsb", bufs=2) as hsbp, \
         tc.tile_pool(name="fco", bufs=3) as fcop, \
         tc.tile_pool(name="gps", bufs=2, space="PSUM") as gps, \
         tc.tile_pool(name="fcps", bufs=2, space="PSUM") as fcps, \
         tc.tile_pool(name="trps", bufs=2, space="PSUM") as trps:

        # ---- constants / small DMAs -----------------------------------
        id128 = constp.tile([P, P], F16)
        make_identity(nc, id128)
        ones128 = constp.tile([1, P], F16)
        nc.gpsimd.memset(ones128, 1.0)
        bsum0_s = constp.tile([1, GSL], F16)
        nc.sync.dma_start(bsum0_s, bsum0)
        bsum1_s = constp.tile([1, GSL], F16)
        nc.sync.dma_start(bsum1_s, bsum1)

        # ---- DMAs needed first: init matmul operands ------------------
        featT_s = resp.tile([P, KF, B], F16)
        nc.sync.dma_start(featT_s, featT_v)
        ihb_s = constp.tile([1, 2 * HSL], F16)
        nc.sync.dma_start(ihb_s, initb_h)
        icb_s = constp.tile([1, 2 * HSL], F16)
        nc.sync.dma_start(icb_s, initb_c)
        initw_s = resp.tile([P, KF, 2, 2 * HSL], F16)   # [.,k,(h|c),cols]
        nc.sync.dma_start(initw_s[:, :, 0, :], initw_h_v)
        nc.sync.dma_start(initw_s[:, :, 1, :], initw_c_v)

        # ---- embedding gathers for blocks 0..2 (gpsimd queue) ---------
        def gather_block(g):
            idx_t = embg.tile([P, 1, 1], mybir.dt.int32, tag="idx")
            nc.sync.dma_start(idx_t, idx_v[:, g:g + 1, :])
            rows = embg.tile([P, E], F16, tag="rows")
            nc.gpsimd.indirect_dma_start(
                out=rows[:], out_offset=None, in_=table[:],
                in_offset=bass.IndirectOffsetOnAxis(ap=idx_t[:, 0, :],
                                                    axis=0),
            )
            return rows

        rows_q = {}
        for g in range(3):
            rows_q[g] = gather_block(g)

        # ---- recurrence weights (needed from iter 0/1) ----------------
        wih0_s = resp.tile([P, KE, GSL], F16)
        nc.sync.dma_start(wih0_s, wih0T_v)
        whh0_s = resp.tile([P, KH, GSL], F16)
        nc.sync.dma_start(whh0_s, whh0T_v)
        wih1_s = resp.tile([P, KH, GSL], F16)
        nc.sync.dma_start(wih1_s, wih1T_v)
        whh1_s = resp.tile([P, KH, GSL], F16)
        nc.sync.dma_start(whh1_s, whh1T_v)

        X0_s = resp.tile([P, T, GSL], F16)     # [b, t, gate-slice]
        c0_s = statep.tile([P, HSL], F32)
        c1_s = statep.tile([P, HSL], F32)
        h1T_init = statep.tile([P, P], F16)    # layer-1 h(-1).T slice

        def do_cc(stg_tile):
            """stg_tile: SBUF [j, 2, b] -> flat AllGather -> [c][j][l][b]."""
            cc_in = ccp.tile([2 * P * P], F16, tag="ccin")
            nc.sync.dma_start(
                cc_in.rearrange("(j l b) -> j l b", j=P, l=2), stg_tile)
            cc_out = ccp.tile([NCORES * 2 * P * P], F16, tag="ccout",
                              addr_space="Shared")
            nc.gpsimd.collective_compute(
                "AllGather", mybir.AluOpType.bypass,
                replica_groups=RG, ins=[cc_in[:]], outs=[cc_out[:]],
            )
            return cc_out

        def load_hg(cc_out):
            # l=0 first so whh0/wih1 (need only h0_full) start earlier.
            hg = hgp.tile([P, KH, 2, P], F16, tag="hg")
            v = cc_out.rearrange("(c j l b) -> j c l b", c=NCORES, j=P, l=2)
            nc.sync.dma_start(hg[:, :, 0, :], v[:, :, 0, :])
            nc.gpsimd.dma_start(hg[:, :, 1, :], v[:, :, 1, :])
            return hg

        def transpose_block(g, rows):
            """rows [r, E] -> embT block [e_part, ke, r] via PE."""
            pt = trps.tile([P, KE, P], F16, tag="pt")
            for ke in range(KE):
                nc.tensor.transpose(pt[:, ke, :], rows[:, ke * P:(ke + 1) * P],
                                    id128)
            ebt = embtp.tile([P, KE, P], F16, tag="ebt")
            nc.vector.tensor_copy(ebt, pt)
            return ebt

        def x0_block(m, ebt):
            """X0[:, m, :] = emb_block_m @ wih0_slice + b0."""
            psx = fcps.tile([P, GSL], F32, tag="fc")
            for ke in range(KE):
                nc.tensor.matmul(psx, ebt[:, ke, :], wih0_s[:, ke, :],
                                 start=(ke == 0), stop=False)
            nc.tensor.matmul(psx, ones128, bsum0_s, start=False, stop=True)
            nc.vector.tensor_copy(X0_s[:, m, :], psx)

        # ---------------- init h0/c0 slices + init CC ------------------
        stg0 = stgp.tile([P, 2, P], F16, tag="stg")
        for which in range(2):
            bias_s = ihb_s if which == 0 else icb_s
            ps = fcps.tile([P, GSL], F32, tag="fc")
            for k in range(KF):
                nc.tensor.matmul(ps[:, 0:2 * HSL], featT_s[:, k, :],
                                 initw_s[:, k, which, :],
                                 start=(k == 0), stop=False)
            nc.tensor.matmul(ps[:, 0:2 * HSL], ones128, bias_s,
                             start=False, stop=True)
            if which == 0:
                hh = hsbp.tile([P, 2 * HSL], F16, tag="hh")
                nc.vector.tensor_copy(hh, ps[:, 0:2 * HSL])
                pt = trps.tile([P, KE, P], F16, tag="pt")
                for lay in range(L):
                    nc.tensor.transpose(pt[:, lay, :],
                                        hh[:, lay * P:(lay + 1) * P], id128)
                nc.vector.tensor_copy(stg0, pt[:, 0:2, :])
                nc.vector.tensor_copy(h1T_init, pt[:, 1, :])
            else:
                nc.vector.tensor_copy(c0_s, ps[:, 0:HSL])
                nc.vector.tensor_copy(c1_s, ps[:, HSL:2 * HSL])
        cc_prev = do_cc(stg0)

        # ---- prologue embT/X0 for steps 0..1 --------------------------
        ebt_q = {}
        for g in range(2):
            ebt_q[g] = transpose_block(g, rows_q.pop(g))
            x0_block(g, ebt_q.pop(g))

        # ---- FC weights (first needed at iter 2) ----------------------
        fcw_s = resp.tile([P, KH, VSL], F16)
        nc.sync.dma_start(fcw_s, fcwT_v)
        fcb_s = resp.tile([P, VSL], F32)
        nc.sync.dma_start(fcb_s, fcb_rep)

        # ---------------- recurrence + interleaved everything ----------
        def lstm_tail(ps, c_s, tagpfx):
            """gates psum [b, i|f|o|g] -> h slice [b, j] f16."""
            sig = actp.tile([P, 3 * HSL], F32, tag=tagpfx + "sig")
            nc.scalar.activation(sig, ps[:, 0:3 * HSL], SIG)
            tg = actp.tile([P, HSL], F32, tag=tagpfx + "tg")
            nc.scalar.activation(tg, ps[:, 3 * HSL:4 * HSL], TANH)
            nc.vector.tensor_mul(c_s, sig[:, HSL:2 * HSL], c_s)
            nc.vector.tensor_mul(tg, sig[:, 0:HSL], tg)
            nc.vector.tensor_add(c_s, c_s, tg)
            tct = actp.tile([P, HSL], F32, tag=tagpfx + "tc")
            nc.scalar.activation(tct, c_s, TANH)
            h_sb = hsbp.tile([P, HSL], F16, tag=tagpfx + "h")
            nc.vector.tensor_mul(h_sb, sig[:, 2 * HSL:3 * HSL], tct)
            return h_sb

        def fc_chunk(t, lo, hi, hg):
            w = hi - lo
            psf = fcps.tile([P, GSL], F32, tag="fc")
            for k in range(KH):
                nc.tensor.matmul(psf[:, :w], hg[:, k, 1, :],
                                 fcw_s[:, k, lo:hi],
                                 start=(k == 0), stop=(k == KH - 1))
            ot = fcop.tile([P, 512], F32, tag="fco")
            nc.vector.tensor_add(ot[:, :w], psf[:, :w], fcb_s[:, lo:hi])
            nc.sync.dma_start(out[t * P:(t + 1) * P, lo:hi], ot[:, :w])

        for i in range(T + 2):           # L0 step i, L1 step i-1, FC i-2
            have_l0 = i < T
            have_l1 = 1 <= i <= T
            have_fc = i >= 2
            if i <= T + 1:
                hg = load_hg(cc_prev)

            if have_l0:
                # X0 add first: no dependency on the gathered h, so the
                # PE issues it while the collective is still in flight.
                ps0 = gps.tile([P, GSL], F32, tag="g0")
                nc.tensor.matmul(ps0, id128, X0_s[:, i, :],
                                 start=True, stop=False)
                for k in range(KH):
                    nc.tensor.matmul(ps0, hg[:, k, 0, :], whh0_s[:, k, :],
                                     start=False, stop=(k == KH - 1))
            if have_l1:
                ps1 = gps.tile([P, GSL], F32, tag="g1")
                nc.tensor.matmul(ps1, ones128, bsum1_s,
                                 start=True, stop=False)
                for k in range(KH):
                    nc.tensor.matmul(ps1, hg[:, k, 0, :], wih1_s[:, k, :],
                                     start=False, stop=False)
                for k in range(KH):
                    nc.tensor.matmul(ps1, hg[:, k, 1, :], whh1_s[:, k, :],
                                     start=False, stop=(k == KH - 1))

            # FC part A fills the PE while ACT/DVE drain the gates.
            if have_fc:
                fc_chunk(i - 2, 0, 512, hg)

            if i <= T:
                stg = stgp.tile([P, 2, P], F16, tag="stg")
                pt = trps.tile([P, KE, P], F16, tag="pt")
                if have_l0:
                    h0_sb = lstm_tail(ps0, c0_s, "l0")
                    nc.tensor.transpose(pt[:, 0, :], h0_sb, id128)
                    nc.vector.tensor_copy(stg[:, 0, :], pt[:, 0, :])
                if have_l1:
                    h1_sb = lstm_tail(ps1, c1_s, "l1")
                    nc.tensor.transpose(pt[:, 1, :], h1_sb, id128)
                    nc.vector.tensor_copy(stg[:, 1, :], pt[:, 1, :])
                    if not have_l0:  # i == T: l0 half unused, fill
                        nc.vector.tensor_copy(stg[:, 0, :], pt[:, 1, :])
                else:                # i == 0: ship init h1 slice
                    nc.vector.tensor_copy(stg[:, 1, :], h1T_init)
                cc_prev = do_cc(stg)

            # ---- collective window fill: FC B/C + embT/X0 pipeline ----
            if have_fc:
                fc_chunk(i - 2, 512, 1024, hg)
            if i + 3 < T:
                rows_q[i + 3] = gather_block(i + 3)
            if i + 2 < T:
                ebt = transpose_block(i + 2, rows_q.pop(i + 2))
                x0_block(i + 2, ebt)
            if have_fc:
                fc_chunk(i - 2, 1024, VSL, hg)

    nc.finalize()
    return nc


def _get_compiled():
    if "nc" not in _cache:
        _cache["nc"] = _build_nc()
    return _cache["nc"]


def _prep_inputs(features, captions, embed_table, init_h_w, init_h_b,
                 init_c_w, init_c_b, w_ih0, w_hh0, b_ih0, b_hh0,
                 w_ih1, w_hh1, b_ih1, b_hh1, fc_w, fc_b):
    f32 = lambda x: np.asarray(x, dtype=np.float32)
    f16 = lambda x: np.ascontiguousarray(np.asarray(x, dtype=np.float32)
                                         ).astype(np.float16)

    w_ih0, w_hh0, w_ih1, w_hh1 = map(f32, (w_ih0, w_hh0, w_ih1, w_hh1))
    init_h_w, init_c_w = f32(init_h_w), f32(init_c_w)
    b0 = f32(b_ih0) + f32(b_hh0)
    b1 = f32(b_ih1) + f32(b_hh1)
    init_h_b, init_c_b = f32(init_h_b), f32(init_c_b)
    fc_w, fc_b = f32(fc_w), f32(fc_b)
    features = f32(features)
    captions = np.asarray(captions).astype(np.int32)

    shared = {
        "table": f16(embed_table),
        "featT": np.ascontiguousarray(features.T).astype(np.float16),
        "emb_idx": np.ascontiguousarray(captions.T.reshape(TB, 1)),
    }

    in_maps = []
    for c in range(NCORES):
        hc = np.arange(c * HSL, (c + 1) * HSL)
        # torch gate order i,f,g,o in rows; our slice order i,f,o,g
        gsel = np.r_[0 * H + hc, 1 * H + hc, 3 * H + hc, 2 * H + hc]
        isel = np.r_[hc * L + 0, hc * L + 1]   # [layer0 block, layer1 block]
        vsl = slice(c * VSL, (c + 1) * VSL)
        m = dict(shared)
        m["whh0T"] = np.ascontiguousarray(w_hh0[gsel].T).astype(np.float16)
        m["wih1T"] = np.ascontiguousarray(w_ih1[gsel].T).astype(np.float16)
        m["whh1T"] = np.ascontiguousarray(w_hh1[gsel].T).astype(np.float16)
        m["wih0T"] = np.ascontiguousarray(w_ih0[gsel].T).astype(np.float16)
        m["bsum0"] = b0[gsel][None, :].astype(np.float16)
        m["bsum1"] = b1[gsel][None, :].astype(np.float16)
        m["initw_h"] = np.ascontiguousarray(init_h_w[isel].T).astype(np.float16)
        m["initw_c"] = np.ascontiguousarray(init_c_w[isel].T).astype(np.float16)
        m["initb_h"] = init_h_b[isel][None, :].astype(np.float16)
        m["initb_c"] = init_c_b[isel][None, :].astype(np.float16)
        m["fcwT"] = np.ascontiguousarray(fc_w[vsl].T).astype(np.float16)
        m["fcb_rep"] = np.ascontiguousarray(
            np.broadcast_to(fc_b[vsl], (P, VSL))).astype(np.float32)
        in_maps.append(m)
    return in_maps


last_results = None


def kernel(**inputs) -> np.ndarray:
    global last_results
    nc = _get_compiled()
    in_maps = _prep_inputs(**inputs)
    res = run_bass_kernel_spmd(nc, in_maps, core_ids=list(range(NCORES)))
    last_results = res
    parts = [res.results[c]["out"].reshape(T, B, VSL) for c in range(NCORES)]
    return np.concatenate(parts, axis=2)
